# revision 1
# baseline (speedup 1.0000x reference)
"""Trainium2 Bass kernel for the CustomRNN problem.

Model (per batch element b):
    u_t = W_in @ x_t + bias + sigma*sqrt(2*alpha) * noise_t          [N=256]
    r_{t+1} = (1-alpha) * r_t + alpha * relu(W_rec @ r_t + u_t)
    out_t = W_out @ r_{t+1} + b_out                                  [3]

Sharding: data-parallel over batch across 8 cores (32 batch each), weights
replicated.

Per-core on-chip layout ("option A"): state H kept as [128 partitions, 2
hidden-chunks x 32 batch cols] fp16 tiles in an SBUF history ring (one tile
per 50-step chunk, which also feeds the batched output projection).  Each
step runs:
  PE : 4 identity-matmuls inject alpha*u_t into PSUM (per group/hidden chunk),
       8 matmuls accumulate alpha*W_rec@r (4 weight chunks x 2 batch groups)
  DVE: per batch group one fused op  H' = max(psum, 0) + Htilde   (relu+EMA)
       plus Htilde' = (1-alpha)*H' off the critical path.
All matmul operands are fp16 (validated: output rel-l2 ~5e-4 vs fp32
reference); PSUM accumulation is fp32.
"""

import numpy as np

import concourse.bacc as bacc
import concourse.mybir as mybir
from concourse.tile import TileContext, add_dep_helper
from concourse.bass_utils import run_bass_kernel_spmd

ALPHA = 0.2
NOISE_SCALE = 0.05 * float(np.sqrt(2 * ALPHA))
N = 256
NCORES = 8
BC = 32          # batch per core
G = 2            # batch groups (pipeline lanes)
GB = BC // G     # 16
F16 = mybir.dt.float16
F32 = mybir.dt.float32

_CACHE = {}


def _dedup_ldweights(nc):
    """Remove InstLdweights that reload the exact weights already resident in
    the PE array (same source AP, no other LDW in between).  Tile's lowering
    emits one LDW per matmul; consecutive same-weight matmuls only need the
    first.  Any semaphore waits parked on a removed LDW migrate to the next
    instruction so no synchronization is lost (bacc's event-semaphore pass
    later re-splits multi-wait instructions as required)."""
    removed = 0
    for bb in nc.m.functions[0].blocks:
        il = bb.instructions
        last_sig = None
        drop = []
        pending = {}  # index -> waits to migrate
        for idx, inst in enumerate(il):
            if inst.__class__.__name__ != "InstLdweights":
                continue
            sig = repr(inst.ins[0])
            if sig == last_sig:
                drop.append(idx)
            else:
                last_sig = sig
        for idx in reversed(drop):
            inst = il[idx]
            waits = list(inst.sync_info.on_wait) if inst.sync_info else []
            ups = list(inst.sync_info.on_update) if inst.sync_info else []
            if ups:
                continue  # updates would be lost; keep this LDW
            if waits:
                # move waits to the following instruction
                nxt = il[idx + 1] if idx + 1 < len(il) else None
                if nxt is None:
                    continue
                si = nxt.sync_info
                nw = (list(si.on_wait) if si else []) + waits
                nu = list(si.on_update) if si else []
                import concourse.mybir as _mb
                nxt.sync_info = _mb.SyncInfo(on_wait=nw, on_update=nu)
            il.pop(idx)
            removed += 1
    return removed


def _build(T, TC, YB, reps=1):
    """Build the Bass program for sequence length T, chunk TC, y-block YB.
    reps>1 repeats the whole computation (timing calibration only)."""
    NCH = T // TC
    assert NCH * TC == T and TC % YB == 0
    nc = bacc.Bacc("TRN2", num_devices=NCORES)

    noise_d = nc.dram_tensor("noiset", [128, T, 2 * BC], F32, kind="ExternalInput")
    xta_d = nc.dram_tensor("xta", [4, T, BC], F16, kind="ExternalInput")
    w4_d = nc.dram_tensor("w4", [128, 512], F16, kind="ExternalInput")
    id_d = nc.dram_tensor("ident", [128, 128], F16, kind="ExternalInput")
    clo_d = nc.dram_tensor("clo", [128, 128], F16, kind="ExternalInput")
    win_d = nc.dram_tensor("win", [4, 256], F16, kind="ExternalInput")
    wout_d = nc.dram_tensor("wout", [128, 6], F16, kind="ExternalInput")
    woutb_d = nc.dram_tensor("woutb", [1, 3], F16, kind="ExternalInput")
    y_d = nc.dram_tensor("y", [3, T, BC], F32, kind="ExternalOutput")

    with TileContext(nc) as tc:
        with (
            tc.tile_pool(name="consts", bufs=1) as consts,
            tc.tile_pool(name="hist", bufs=2) as histp,
            tc.tile_pool(name="noise", bufs=2) as noisep,
            tc.tile_pool(name="xtap", bufs=2) as xtap,
            tc.tile_pool(name="upp", bufs=2) as upp,
            tc.tile_pool(name="ysbp", bufs=2) as ysbp,
            tc.tile_pool(name="pv", bufs=4, space="PSUM") as pvp,
            tc.tile_pool(name="pxw", bufs=2, space="PSUM") as pxwp,
            tc.tile_pool(name="pyp", bufs=2, space="PSUM") as pyp,
        ):
            w4_sb = consts.tile_from(w4_d[:, :])
            c_sb = consts.tile_from(id_d[:, :])   # fp16(1-alpha) * I
            clo_sb = consts.tile_from(clo_d[:, :])  # low bits of (1-alpha)*I
            win_sb = consts.tile_from(win_d[:, :])
            wout_sb = consts.tile_from(wout_d[:, :])
            woutb_sb = consts.tile_from(woutb_d[:, :])
            scratch = consts.tile([1, 4], F32)
            ones_sb = consts.tile([1, YB * BC], F16)
            nc.vector.memset(ones_sb[:], 1.0)

            # Ordering-only (nosync) chain over every PE matmul: pins the
            # scheduler to the emission order so same-weight matmuls stay
            # adjacent and the LDW dedup pass can collapse their reloads.
            _prev_mm = [None]

            def mm(*args, **kw):
                inst = nc.tensor.matmul(*args, **kw)
                raw = getattr(inst, "ins", inst)
                if _prev_mm[0] is not None:
                    add_dep_helper(raw, _prev_mm[0], sync=False,
                                   reason="pe-stream-order")
                _prev_mm[0] = raw
                return inst

            for rep in range(reps):
              prev_hist = None
              for ck in range(NCH):
                ts0 = ck * TC
                noise_sb = noisep.tile([128, TC, 2 * BC], F32)
                nc.sync.dma_start(out=noise_sb[:], in_=noise_d[:, ts0:ts0 + TC, :])
                xta_sb = xtap.tile([4, TC, BC], F16)
                nc.sync.dma_start(out=xta_sb[:], in_=xta_d[:, ts0:ts0 + TC, :])
                # hist slot s holds state r_{ts0+s}; slot 0 = carry-in
                hist = histp.tile([128, TC + 1, 2, BC], F16)
                up_sb = upp.tile([128, TC, 2, BC], F16)
                ysb = ysbp.tile([3, TC, BC], F32)
                if ck == 0:
                    nc.vector.memset(hist[:, 0], 0.0)
                noise_r = noise_sb[:].rearrange("p t (c b) -> p t c b", c=2)
                # fence: absorb the DMA-queue wait on DVE so the custom STT
                # ops below only ever carry a single (PE) semaphore wait
                nc.vector.tensor_copy(scratch[0:1, 0:1], noise_sb[0:1, 0:1, 0:1])
                # fence for the WAR dep on the ysb slot (output DMA 2 chunks ago)
                nc.vector.memset(ysb[0:1, 0:1, 0:1], 0.0)

                # ---- drive phase: up = (alpha/(1-a))*(W_in x + bias + s*noise)
                for m_c in range(2):
                    for blk in range(TC // YB):
                        pxw = pxwp.tile([128, YB, BC], F32)
                        mm(pxw[:],
                           win_sb[:, m_c * 128:(m_c + 1) * 128],
                           xta_sb[:, blk * YB:(blk + 1) * YB, :],
                           start=True, stop=True)
                        nc.vector.scalar_tensor_tensor(
                            out=up_sb[:, blk * YB:(blk + 1) * YB, m_c, :],
                            in0=noise_r[:, blk * YB:(blk + 1) * YB, m_c, :],
                            scalar=ALPHA * NOISE_SCALE / float(np.float16(1.0 - ALPHA)),
                            in1=pxw[:],
                            op0=mybir.AluOpType.mult,
                            op1=mybir.AluOpType.add,
                        )

                # ---- recurrence (+ inline output blocks every YB passes)
                for l in range(TC):
                    if l == 0 and ck > 0:
                        rd, rs = prev_hist, TC
                    else:
                        rd, rs = hist, l
                    # PSUM accumulates S1 = alpha*(W_rec r + u) + (1-alpha)*r
                    # (the decay rides on the W4 diagonals + C_lo correction).
                    # Then H' = relu(alpha(Wr+u)) + (1-alpha)r
                    #         = max(S1, (1-alpha)r)  — one psum operand only.
                    pv = [pvp.tile([128, 2, GB], F32, tag="pv", name=f"pv{g}")
                          for g in range(G)]
                    # group order alternates per pass so each group's state
                    # update hides behind the other group's matmul run
                    go = (0, 1) if l % 2 == 0 else (1, 0)
                    # u-inject: H-independent, fills the PE while the previous
                    # pass's state updates propagate.  start=True only on each
                    # bank's FIRST matmul (start marks the bank pending-zero).
                    for g in go:
                        gsl = slice(g * GB, (g + 1) * GB)
                        for m_c in range(2):
                            mm(pv[g][:, m_c], c_sb[:],
                               up_sb[:, l, m_c, gsl],
                               start=(m_c == 0), stop=False,
                               skip_group_check=True)
                    for g in go:
                        gsl = slice(g * GB, (g + 1) * GB)
                        for m_c in range(2):
                            mm(pv[g][:, m_c], clo_sb[:],
                               rd[:, rs, m_c, gsl],
                               start=False, stop=False,
                               skip_group_check=True)
                    # alpha * W_rec @ r + decay (4 chunks; both groups share
                    # each chunk's single weight load)
                    for k_c in range(2):
                        for m_c in range(2):
                            for g in go:
                                mm(pv[g][:, m_c],
                                   w4_sb[:, (2 * k_c + m_c) * 128:(2 * k_c + m_c + 1) * 128],
                                   rd[:, rs, k_c, g * GB:(g + 1) * GB],
                                   start=False, stop=(k_c == 1 and m_c == 1),
                                   skip_group_check=True)
                    for g in go:
                        gsl = slice(g * GB, (g + 1) * GB)
                        # H' = max((1-alpha)*H, S1)   (single fused DVE op)
                        nc.vector.scalar_tensor_tensor(
                            out=hist[:, l + 1, :, gsl],
                            in0=rd[:, rs, :, gsl],
                            scalar=1.0 - ALPHA,
                            in1=pv[g][:],
                            op0=mybir.AluOpType.mult,
                            op1=mybir.AluOpType.max)

                    # ---- output projection for each completed YB-step block
                    if (l + 1) % YB == 0:
                        j = l // YB
                        py = pyp.tile([3, YB, BC], F32)
                        for k_c in range(2):
                            mm(py[:],
                               wout_sb[:, k_c * 3:(k_c + 1) * 3],
                               hist[:, 1 + j * YB:1 + (j + 1) * YB, k_c, :],
                               start=(k_c == 0), stop=False,
                               skip_group_check=True)
                        # bias via rank-1 matmul (K=1, ones moving operand)
                        mm(py[:], woutb_sb[:, :],
                           ones_sb[:].rearrange("p (t b) -> p t b", t=YB),
                           start=False, stop=True, skip_group_check=True)
                        nc.scalar.copy(ysb[:, j * YB:(j + 1) * YB, :], py[:])
                nc.sync.dma_start(out=y_d[:, ts0:ts0 + TC, :], in_=ysb[:])
                prev_hist = hist
    _dedup_ldweights(nc)
    nc.finalize()
    return nc


def get_nc(T=1000, TC=50, YB=10, reps=1):
    key = (T, TC, YB, reps)
    if key not in _CACHE:
        _CACHE[key] = _build(T, TC, YB, reps)
    return _CACHE[key]


def make_inputs(x, noise, W_in, W_rec, W_out_w, W_out_b, bias):
    """Host-side shard + layout prep.  Returns in_maps for 8 cores."""
    x = np.asarray(x, np.float32)
    noise = np.asarray(noise, np.float32)
    W_in = np.asarray(W_in, np.float32)
    W_rec = np.asarray(W_rec, np.float32)
    W_out_w = np.asarray(W_out_w, np.float32)
    W_out_b = np.asarray(W_out_b, np.float32)
    bias = np.asarray(bias, np.float32)
    B, T, _ = x.shape

    # W4 chunks carry the state decay on their diagonal: W_rec's diagonal is
    # zero, so chunk (k,k)'s diagonal becomes fp16(1-alpha) exactly.
    decay_hi = float(np.float16(1.0 - ALPHA))          # 0.7998046875
    decay_lo = (1.0 - ALPHA) - decay_hi                # 1.953125e-4
    w4 = np.empty((128, 512), np.float16)
    wrt = ALPHA * W_rec.T + decay_hi * np.eye(256, dtype=np.float32)
    wrt = wrt.astype(np.float16)                       # [k, m]
    for k_c in range(2):
        for m_c in range(2):
            w4[:, (2 * k_c + m_c) * 128:(2 * k_c + m_c + 1) * 128] = \
                wrt[128 * k_c:128 * (k_c + 1), 128 * m_c:128 * (m_c + 1)]
    ident = (decay_hi * np.eye(128)).astype(np.float16)
    clo = (decay_lo * np.eye(128)).astype(np.float16)
    # u injected through C_hi weights -> scale compensated exactly on host
    s = ALPHA / decay_hi
    win = np.empty((4, 256), np.float16)
    win[:3] = (s * W_in.T).astype(np.float16)
    win[3] = (s * bias).astype(np.float16)
    wout = np.empty((128, 6), np.float16)
    wt = W_out_w.T.astype(np.float16)              # [n, 3]
    for k_c in range(2):
        wout[:, 3 * k_c:3 * (k_c + 1)] = wt[128 * k_c:128 * (k_c + 1)]
    woutb = W_out_b.reshape(1, 3).astype(np.float16)

    in_maps = []
    for c in range(NCORES):
        b0 = c * BC
        nz = noise[b0:b0 + BC]                     # [32, T, 256]
        nzt = np.ascontiguousarray(
            nz.reshape(BC, T, 2, 128).transpose(3, 1, 2, 0)).reshape(128, T, 2 * BC)
        xc = x[b0:b0 + BC]                         # [32, T, 3]
        xta = np.empty((4, T, BC), np.float16)
        xta[:3] = xc.transpose(2, 1, 0).astype(np.float16)
        xta[3] = 1.0
        in_maps.append({
            "noiset": nzt, "xta": xta, "w4": w4, "ident": ident, "clo": clo,
            "win": win, "wout": wout, "woutb": woutb,
        })
    return in_maps


def gather_output(results, B, T):
    out = np.empty((B, T, 3), np.float32)
    for c in range(NCORES):
        out[c * BC:(c + 1) * BC] = results[c]["y"].transpose(2, 1, 0)
    return out


def kernel(x, noise, W_in, W_rec, W_out_w, W_out_b, bias):
    x = np.asarray(x, np.float32)
    B, T, _ = x.shape
    nc = get_nc(T=T)
    in_maps = make_inputs(x, noise, W_in, W_rec, W_out_w, W_out_b, bias)
    res = run_bass_kernel_spmd(nc, in_maps, list(range(NCORES)))
    return gather_output(res.results, B, T)

